# revision 1
# baseline (speedup 1.0000x reference)
"""Trainium2 Bass kernel for the Dale_CB_STP recurrent cell.

Contract: kernel(**inputs) takes the FULL unsharded inputs (as produced by
reference.setup_inputs()) and returns the FULL [B, NC] output.

Strategy (data-parallel over batch):
  - B=256 is sharded 8 ways -> 32 batch elements per NeuronCore.
  - State lives packed as [128 partitions, 4*32] where
    tile[p, c*32+j] = state[h = c*128 + p, batch j]; v is fp32, the X/U
    gating state is kept in bf16 in precomputed affine form
    (XU = [Xn|Un], BE = [Xn*Un | Ucap*Un-Ucap], AC = [A|C]) so the
    per-step critical chain is only sigma -> tp -> XU' -> s2 -> s.
  - Per step two h x h matmuls (Ksp@r for the z-gate, DT*W@s for the v
    update) run with bf16 stationary weight tiles (FWL) and bf16 moving
    activations; P_z@x_t and DT*P@x_t are folded into the same PSUM
    accumulation groups, b_z is folded into the sigmoid bias (per-partition),
    DT*b_v into a constant tile add.
  - The Un/Xn clip against [Ucap, 1] is mathematically inactive (proved from
    the update equations given 0<r<1, Ucap<=U<=1) and is dropped.
  - No cross-core communication; host gathers the 8 [32,10] outputs.
"""

import sys

import numpy as np

for _p in ("/opt/trn_rl_repo",):
    if _p not in sys.path:
        sys.path.insert(0, _p)

H, IN, B, T, NCLS = 512, 128, 256, 256, 10
Z_MIN, Z_MAX, DT = 0.001, 0.1, 0.1
N_CORES = 8
BL = B // N_CORES  # 32
NCH = H // 128  # 4 h-chunks
UNROLL = 32

PROFILE = False
TRACE_DIR = None

_cache = {}


def _build_nc():
    import concourse.bacc as bacc
    import concourse.bass as bass
    import concourse.tile as tile
    from concourse import mybir

    f32 = mybir.dt.float32
    bf16 = mybir.dt.bfloat16
    Alu = mybir.AluOpType
    Act = mybir.ActivationFunctionType

    nc = bacc.Bacc("TRN2", target_bir_lowering=False, debug=False, num_devices=1)

    # ---- DRAM I/O ----
    xT = nc.dram_tensor("xT", [IN, T * BL], f32, kind="ExternalInput").ap()
    KT = nc.dram_tensor("KT", [H, H], f32, kind="ExternalInput").ap()
    CT = nc.dram_tensor("CT", [H, H], f32, kind="ExternalInput").ap()
    PT = nc.dram_tensor("PT", [IN, H], f32, kind="ExternalInput").ap()
    PzT = nc.dram_tensor("PzT", [IN, H], f32, kind="ExternalInput").ap()
    cvec = nc.dram_tensor("cvec", [H, 5], f32, kind="ExternalInput").ap()
    ev = nc.dram_tensor("ev", [1, 2], f32, kind="ExternalInput").ap()
    fcwT = nc.dram_tensor("fcwT", [H // 2, NCLS], f32, kind="ExternalInput").ap()
    fcb = nc.dram_tensor("fcb", [1, NCLS], f32, kind="ExternalInput").ap()
    ind = nc.dram_tensor("ind", [128, 128], f32, kind="ExternalInput").ap()
    bzin = nc.dram_tensor("bzin", [128, 128], f32, kind="ExternalInput").ap()
    bvin = nc.dram_tensor("bvin", [128, 128], f32, kind="ExternalInput").ap()
    out = nc.dram_tensor("out", [BL, NCLS], f32, kind="ExternalOutput").ap()

    with tile.TileContext(nc) as tc:
        _trace(tc, nc, bass, mybir, f32, bf16, Alu, Act,
               xT, KT, CT, PT, PzT, cvec, ev, fcwT, fcb, ind, bzin, bvin, out)

    nc.compile()
    return nc


def _trace(tc, nc, bass, mybir, f32, bf16, Alu, Act,
           xT, KT, CT, PT, PzT, cvec, ev, fcwT, fcb, ind, bzin, bvin, out):
    from contextlib import ExitStack

    from concourse.tile import add_dep_helper

    ds = bass.ds
    SIG = Act.Sigmoid

    ctx = ExitStack()
    const = ctx.enter_context(tc.tile_pool(name="const", bufs=1))
    psum = ctx.enter_context(tc.tile_pool(name="psum", bufs=1, space="PSUM"))

    # ---------------- one-time prep ----------------
    # e_e / e_i broadcast to [128,1] then scaled by DT
    e_bc = const.tile([128, 2], f32, name="e_bc")
    nc.sync.dma_start(
        out=e_bc,
        in_=bass.AP(tensor=ev.tensor, offset=ev.offset, ap=[[0, 128], [1, 2]]),
    )
    edt = const.tile([128, 2], f32, name="edt")
    nc.vector.tensor_scalar(edt, e_bc, float(DT), None, Alu.mult)

    # weight strips
    kspbf = []  # Ksp.T strips [128(k), 512(m)] bf16, per k-chunk
    wdtbf = []  # (DT * W).T strips bf16
    with tc.tile_pool(name="stage", bufs=2) as stage:
        for kc in range(NCH):
            kt_s = stage.tile([128, H], f32, tag="kt")
            ct_s = stage.tile([128, H], f32, tag="ct")
            nc.sync.dma_start(kt_s, KT[128 * kc:128 * (kc + 1), :])
            nc.sync.dma_start(ct_s, CT[128 * kc:128 * (kc + 1), :])
            # softplus(x) = ln(1 + exp(x)); inputs are in [0, ~0.05] so no
            # overflow concerns (Softplus has no ACT table in this build)
            ksp_f = stage.tile([128, H], f32, tag="kspf")
            csp_f = stage.tile([128, H], f32, tag="cspf")
            nc.scalar.activation(ksp_f, kt_s, Act.Exp)
            nc.scalar.activation(csp_f, ct_s, Act.Exp)
            nc.vector.tensor_scalar(ksp_f, ksp_f, 1.0, None, Alu.add)
            nc.vector.tensor_scalar(csp_f, csp_f, 1.0, None, Alu.add)
            nc.scalar.activation(ksp_f, ksp_f, Act.Ln)
            nc.scalar.activation(csp_f, csp_f, Act.Ln)
            kbf = const.tile([128, H], bf16, name=f"kspbf{kc}")
            nc.vector.tensor_copy(kbf, ksp_f)
            kspbf.append(kbf)
            w_f = stage.tile([128, H], f32, tag="wf")
            nc.vector.tensor_tensor(w_f, ksp_f, csp_f, Alu.add)
            wbf = const.tile([128, H], bf16, name=f"wdtbf{kc}")
            e_col = edt[:, 0:1] if kc < NCH // 2 else edt[:, 1:2]
            nc.vector.tensor_scalar(wbf, w_f, e_col, None, Alu.mult)
            wdtbf.append(wbf)

        pz_bf = const.tile([128, H], bf16, name="pz_bf")
        pdt_bf = const.tile([128, H], bf16, name="pdt_bf")
        p_s = stage.tile([128, H], f32, tag="ps")
        pz_s = stage.tile([128, H], f32, tag="pzs")
        nc.sync.dma_start(p_s, PT)
        nc.sync.dma_start(pz_s, PzT)
        nc.vector.tensor_copy(pz_bf, pz_s)
        nc.vector.tensor_scalar(pdt_bf, p_s, float(DT), None, Alu.mult)

        # x: load fp32, cast whole thing to bf16 resident
        x_bf = const.tile([128, T * BL], bf16, name="x_bf")
        NXC = 8
        xw = T * BL // NXC
        for i in range(NXC):
            x_s = stage.tile([128, xw], f32, tag="xs")
            nc.sync.dma_start(x_s, xT[:, i * xw:(i + 1) * xw])
            nc.vector.tensor_copy(x_bf[:, i * xw:(i + 1) * xw], x_s)

    # per-chunk [128,1] constant vectors
    cv = []
    bz_c, zx_c, zu_c, uc_c, c1x_c, cB_c, bvdt_c = [], [], [], [], [], [], []
    caz_c, nuc_c = [], []
    for c in range(NCH):
        t_cv = const.tile([128, 5], f32, name=f"cv{c}")
        nc.sync.dma_start(t_cv, cvec[128 * c:128 * (c + 1), :])
        cv.append(t_cv)
        sx = const.tile([128, 3], f32, name=f"sig{c}")
        nc.scalar.activation(sx[:, 0:1], t_cv[:, 0:1], SIG)
        nc.scalar.activation(sx[:, 1:2], t_cv[:, 1:2], SIG)
        nc.scalar.activation(sx[:, 2:3], t_cv[:, 2:3], SIG)
        dv = const.tile([128, 6], f32, name=f"dv{c}")
        # dv cols: 0=z_x 1=z_u 2=Ucap 3=(1-z_x) 4=(1-z_u) 5=DT*b_v
        nc.vector.tensor_scalar(dv[:, 0:1], sx[:, 0:1], float(Z_MAX - Z_MIN),
                                float(Z_MIN), Alu.mult, Alu.add)
        nc.vector.tensor_scalar(dv[:, 1:2], sx[:, 1:2], float(Z_MAX - Z_MIN),
                                float(Z_MIN), Alu.mult, Alu.add)
        nc.vector.tensor_scalar(dv[:, 2:3], sx[:, 2:3], 0.9, None, Alu.mult)
        nc.vector.tensor_scalar(dv[:, 3:4], dv[:, 0:1], -1.0, 1.0,
                                Alu.mult, Alu.add)
        nc.vector.tensor_scalar(dv[:, 4:5], dv[:, 1:2], -1.0, 1.0,
                                Alu.mult, Alu.add)
        nc.vector.tensor_scalar(dv[:, 5:6], dv[:, 2:3], dv[:, 1:2], None,
                                Alu.mult)  # caz = Ucap*z_u
        ex = const.tile([128, 1], f32, name=f"nuc{c}")
        nc.vector.tensor_scalar(ex, dv[:, 2:3], -1.0, None, Alu.mult)
        nuc_c.append(ex)
        caz_c.append(dv[:, 5:6])
        zx_c.append(dv[:, 0:1])
        zu_c.append(dv[:, 1:2])
        uc_c.append(dv[:, 2:3])
        c1x_c.append(dv[:, 3:4])
        cB_c.append(dv[:, 4:5])
        bz_c.append(t_cv[:, 3:4])

    # expanded [128, 128] constant tiles (chunk vec broadcast over 32 cols)
    ones_t = const.tile([128, 128], f32, name="ones_t")
    nc.vector.memset(ones_t, 1.0)

    def expand(vecs, name):
        e = const.tile([128, 128], f32, name=name)
        for c in range(NCH):
            sl = slice(32 * c, 32 * (c + 1))
            nc.vector.tensor_scalar(e[:, sl], ones_t[:, sl], vecs[c], None,
                                    Alu.mult)
        return e

    def expand2(vl, vr, name):
        e = const.tile([128, 2, 128], f32, name=name)
        for c in range(NCH):
            sl = slice(32 * c, 32 * (c + 1))
            nc.vector.tensor_scalar(e[:, 0, sl], ones_t[:, sl], vl[c], None,
                                    Alu.mult)
            nc.vector.tensor_scalar(e[:, 1, sl], ones_t[:, sl], vr[c], None,
                                    Alu.mult)
        return e

    uc_t = expand(uc_c, "uc_t")
    c1xcB_t = expand2(c1x_c, cB_c, "c1xcB_t")   # [ (1-z_x) | (1-z_u) ]
    zxcaz_t = expand2(zx_c, caz_c, "zxcaz_t")   # [ z_x | Ucap*z_u ]

    # bias rows for the indicator-matmul bias fold:
    #   psum[p, c*32+j] += bias4[c, p] via lhsT=bias4 [K=4,M=128], rhs=ind [4,128]
    ind_t = const.tile([128, 128], bf16, name="ind_t")
    bz4 = const.tile([128, 128], bf16, name="bz4")
    bv4 = const.tile([128, 128], bf16, name="bv4")
    with tc.tile_pool(name="bstage", bufs=1) as bstage:
        ind_s = bstage.tile([128, 128], f32, tag="inds")
        bz_s = bstage.tile([128, 128], f32, tag="bzs")
        bv_s = bstage.tile([128, 128], f32, tag="bvs")
        nc.sync.dma_start(ind_s, ind)
        nc.sync.dma_start(bz_s, bzin)
        nc.sync.dma_start(bv_s, bvin)
        nc.vector.tensor_copy(ind_t, ind_s)
        nc.vector.tensor_copy(bz4, bz_s)
        nc.vector.tensor_scalar(bv4, bv_s, float(DT), None, Alu.mult)

    # fc weights
    fcw_sb = const.tile([128, 2, NCLS], f32, name="fcw_sb")
    nc.sync.dma_start(fcw_sb[:, 0, :], fcwT[0:128, :])
    nc.sync.dma_start(fcw_sb[:, 1, :], fcwT[128:256, :])
    fcb_sb = const.tile([1, NCLS], f32, name="fcb_sb")
    nc.sync.dma_start(fcb_sb, fcb)

    # ---------------- state tiles (parity ping-pong) ----------------
    # XU[i]: [Xn | Un] packed [128, 2, 128] bf16
    # BE[i]: [ -Un*Xn | Ucap*(1-Un) ] bf16 (precomputed off-chain)
    # AC[i]: [ z_x+(1-z_x)X | Ucap*z_u+(1-z_u)U ] bf16 (off-chain)
    def pair(shape, name, dt):
        return [const.tile(shape, dt, name=f"{name}{i}") for i in (0, 1)]

    XU = pair([128, 2, 128], "XU", bf16)
    BE = pair([128, 2, 128], "BE", bf16)
    AC = pair([128, 2, 128], "AC", bf16)
    tp_b = pair([128, 2, 128], "tp", bf16)
    acp_b = pair([128, 2, 128], "acp", bf16)
    vb = [psum.tile([128, 128], f32, name=f"vst{i}") for i in (0, 1)]
    r_b = pair([128, 128], "r", bf16)
    g_b = pair([128, 128], "g", f32)
    s2_b = pair([128, 128], "s2", bf16)
    sbf_b = pair([128, 128], "sbf", bf16)
    f_b = pair([128, 128], "f", f32)
    v1_b = pair([128, 128], "v1", f32)
    e1_b = pair([128, 128], "e1", bf16)

    psz = [psum.tile([128, 128], f32, name=f"psz{i}") for i in (0, 1)]
    psv = [psum.tile([128, 128], f32, name=f"psv{i}") for i in (0, 1)]

    def off_chain(i, after=None):
        """From XU[i], compute BE[i] and AC[i] (for the NEXT step).

        BE = [ Xn*Un | Ucap*Un - Ucap ]  so that  XU' = AC - BE*r  gives
        Xn' = A - Xn*Un*r  and  Un' = C + Ucap*(1-Un)*r.
        BE[:,0,:] (= Xn*Un) is written by the on-chain s2 op in step().
        """
        xu, be, ac, acp = XU[i], BE[i], AC[i], acp_b[i]
        # E' = (Un - 1) * Ucap in a single fused DVE op
        nc.vector.scalar_tensor_tensor(be[:, 1, :], xu[:, 1, :], 1.0, uc_t,
                                       Alu.subtract, Alu.mult)
        # [a|c] = [c1x|cB] * XU ; AC = that + [zx|caz]
        i1 = nc.vector.tensor_tensor(acp, c1xcB_t, xu, Alu.mult)
        i2 = nc.vector.tensor_tensor(ac, acp, zxcaz_t, Alu.add)
        if after is not None:
            # keep DVE off-chain work out of the critical s-chain window
            add_dep_helper(i1.ins, after.ins, sync=False,
                           reason="off-chain after sbf")

    # init: X=1, U=Ucap, v=0 into parity set 0 (read at u=0)
    nc.vector.memset(XU[0][:, 0, :], 1.0)
    nc.vector.tensor_copy(XU[0][:, 1, :], uc_t)
    nc.vector.memset(vb[0], 0.0)
    nc.vector.tensor_tensor(BE[0][:, 0, :], XU[0][:, 0, :], XU[0][:, 1, :],
                            Alu.mult)
    off_chain(0)

    # ---------------- the scan ----------------
    def step(rd, wr, pp, xt):
        v, vn = vb[rd], vb[wr]
        r, g = r_b[rd], g_b[rd]
        tp, sbf = tp_b[rd], sbf_b[rd]
        f, v1 = f_b[rd], v1_b[rd]
        pz, pv = psz[pp], psv[pp]
        xu_n = XU[wr]

        nc.scalar.activation(r, v, SIG)  # bf16 out; feeds MMs and state math
        # critical chain (all DVE): tp = BE*r ; XU' = AC + tp ; s2 ; s
        r2 = bass.AP(tensor=r.tensor, offset=r.offset,
                     ap=[r.ap[0], [0, 2], r.ap[1]])
        nc.vector.tensor_tensor(tp, BE[rd], r2, Alu.mult)
        nc.vector.tensor_tensor(xu_n, AC[rd], tp, Alu.subtract)
        s2 = BE[wr][:, 0, :]  # doubles as next step's B = Xn*Un
        nc.vector.tensor_tensor(s2, xu_n[:, 0, :], xu_n[:, 1, :], Alu.mult)
        sbf_i = nc.vector.tensor_tensor(sbf, s2, r, Alu.mult)

        # z matmuls: K=128-padded bias matmul initializes the whole tile,
        # then per m-slice: += sum_k KspT[k][:,m] @ r[k] + PzT[m] @ xt
        nc.tensor.matmul(pz, bz4, ind_t, start=True, stop=False,
                         skip_group_check=True)
        for m in range(NCH):
            osl = pz[:, 32 * m:32 * (m + 1)]
            msl = slice(128 * m, 128 * (m + 1))
            for kc in range(NCH):
                nc.tensor.matmul(osl, kspbf[kc][:, msl],
                                 r[:, 32 * kc:32 * (kc + 1)],
                                 start=False, stop=False, skip_group_check=True)
            nc.tensor.matmul(osl, pz_bf[:, msl], xt, start=False,
                             stop=(m == NCH - 1), skip_group_check=True)
        nc.scalar.activation(g, pz, SIG)

        # v matmuls
        nc.tensor.matmul(pv, bv4, ind_t, start=True, stop=False,
                         skip_group_check=True)
        for m in range(NCH):
            osl = pv[:, 32 * m:32 * (m + 1)]
            msl = slice(128 * m, 128 * (m + 1))
            for kc in range(NCH):
                nc.tensor.matmul(osl, wdtbf[kc][:, msl],
                                 sbf[:, 32 * kc:32 * (kc + 1)],
                                 start=False, stop=False, skip_group_check=True)
            nc.tensor.matmul(osl, pdt_bf[:, msl], xt, start=False,
                             stop=(m == NCH - 1), skip_group_check=True)

        nc.scalar.activation(f, g, Act.Identity, bias=1.0, scale=-float(DT))
        nc.vector.tensor_tensor(v1, f, v, Alu.mult)
        nc.vector.tensor_tensor(vn, v1, pv, Alu.add)

        # off-chain state-prep for the next step; scheduler fills MM gaps
        off_chain(wr, after=sbf_i)

    with tc.For_i(0, T * BL, UNROLL * BL, staggered_reset=True,
                  hint_engines=(mybir.EngineType.PE, mybir.EngineType.DVE,
                                mybir.EngineType.Activation,
                                mybir.EngineType.Pool)) as tb:
        for u in range(UNROLL):
            step(u % 2, 1 - u % 2, u % 2, x_bf[:, ds(tb + u * BL, BL)])

    # ---------------- final fc ----------------
    # after T steps (T % 2 == 0) the live state is parity 0
    vf = const.tile([128, 64], f32, name="vf_sb")
    nc.vector.tensor_copy(vf, vb[0][:, 0:64])
    ps_fc = psum.tile([BL, NCLS], f32, name="ps_fc")
    nc.tensor.matmul(ps_fc, vf[:, 0:32], fcw_sb[:, 0, :], start=True, stop=False)
    nc.tensor.matmul(ps_fc, vf[:, 32:64], fcw_sb[:, 1, :], start=False, stop=False)
    nc.tensor.matmul(ps_fc, ones_t[0:1, 0:BL], fcb_sb, start=False, stop=True)
    out_s = const.tile([BL, NCLS], f32, name="out_s")
    nc.vector.tensor_copy(out_s, ps_fc)
    nc.sync.dma_start(out, out_s)


def _prep_inputs(inputs):
    x = np.asarray(inputs["x"], np.float32)
    K = np.asarray(inputs["K"], np.float32)
    C = np.asarray(inputs["C"], np.float32)
    P = np.asarray(inputs["P"], np.float32)
    Pz = np.asarray(inputs["P_z"], np.float32)
    cvec = np.stack([
        np.asarray(inputs["c_x"], np.float32)[:, 0],
        np.asarray(inputs["c_u"], np.float32)[:, 0],
        np.asarray(inputs["c_U"], np.float32)[:, 0],
        np.asarray(inputs["b_z"], np.float32)[:, 0],
        np.asarray(inputs["b_v"], np.float32)[:, 0],
    ], axis=1)  # [H, 5]
    ev = np.array([[float(np.asarray(inputs["e_e"]).reshape(-1)[0]),
                    float(np.asarray(inputs["e_i"]).reshape(-1)[0])]], np.float32)
    fcwT = np.ascontiguousarray(
        np.asarray(inputs["fc_w"], np.float32)[:, :H // 2].T)  # [256, 10]
    fcb = np.asarray(inputs["fc_b"], np.float32).reshape(1, NCLS)

    ind = np.zeros((128, 128), np.float32)
    for c in range(NCH):
        ind[c, 32 * c:32 * (c + 1)] = 1.0
    bzin = np.zeros((128, 128), np.float32)
    bzin[:NCH, :] = np.asarray(inputs["b_z"], np.float32)[:, 0].reshape(NCH, 128)
    bvin = np.zeros((128, 128), np.float32)
    bvin[:NCH, :] = np.asarray(inputs["b_v"], np.float32)[:, 0].reshape(NCH, 128)
    shared = {
        "ind": ind, "bzin": bzin, "bvin": bvin,
        "KT": np.ascontiguousarray(K.T),
        "CT": np.ascontiguousarray(C.T),
        "PT": np.ascontiguousarray(P.T),
        "PzT": np.ascontiguousarray(Pz.T),
        "cvec": cvec, "ev": ev, "fcwT": fcwT, "fcb": fcb,
    }
    # x [B, T, IN] -> per core [IN, T*BL]: xT[k, t*BL+j] = x[b0+j, t, k]
    xt_all = np.ascontiguousarray(x.transpose(2, 1, 0))  # [IN, T, B]
    in_maps = []
    for i in range(N_CORES):
        m = dict(shared)
        m["xT"] = np.ascontiguousarray(
            xt_all[:, :, i * BL:(i + 1) * BL]).reshape(IN, T * BL)
        in_maps.append(m)
    return in_maps


def kernel(**inputs):
    from concourse.bass_utils import run_bass_kernel_spmd

    if "nc" not in _cache:
        _cache["nc"] = _build_nc()
    nc = _cache["nc"]
    in_maps = _prep_inputs(inputs)
    kw = {}
    if PROFILE:
        kw = dict(trace=True, tmpdir=TRACE_DIR)
    res = run_bass_kernel_spmd(nc, in_maps, list(range(N_CORES)), **kw)
    if PROFILE:
        _cache["last_result"] = res
    out = np.concatenate([r["out"] for r in res.results], axis=0)
    return out.astype(np.float32)

